# revision 8
# baseline (speedup 1.0000x reference)
"""Multi-head attention (16 heads, d_model=1024, S=2048) on 8 Trainium2 cores.

Sharding: tensor-parallel over heads — each core owns 2 heads (its slice of
Wq/Wk/Wv and the matching 128 columns of Q/K/V and of Wo).  Each core computes
its heads' attention and a row-parallel partial of the final linear; the host
sums the 8 partials and adds bo (the unshard step for row-parallel).

Device-side layout ("T-space"): activations are kept transposed, [feature,
seq], so that
  - projections contract the input feature dim (on partitions),
  - scores^T[t, s] = k_chunk.T @ qT needs no transpose of the attention matrix,
  - softmax denominators come free via a ones-column appended to V,
  - attn@V and the final linear consume exp(scores^T) chunks directly as the
    moving operand at full PE rate (f32r, N=512).
Only the raw Q/K/V input slices (and V again, post-projection, with the ones
row) are transposed, via cheap 128-wide PE transposes.  Softmax is computed
without max-subtraction: scores here are bounded (|s| < ~10), far from fp32
exp overflow, and softmax is shift-invariant.

Matmul operands use float32r (full-rate fp32 streaming, TF32-grade rounding,
~1e-4 relative per matmul).  Set MHA_PREC=f32 for exact-fp32 matmuls
(4x slower on the PE).
"""

import os
import sys

for _p in ("/opt/trn_rl_repo",):
    if _p not in sys.path:
        sys.path.insert(0, _p)

from contextlib import ExitStack

import numpy as np

import concourse.bass as bass
import concourse.tile as tile
from concourse import bacc, mybir
from concourse.bass import ts
from concourse.bass_utils import run_bass_kernel_spmd
from concourse.masks import make_identity

S = 2048          # sequence length
DK = 1024         # d_model
H = 16            # heads
DH = 64           # head dim
NCORES = 8
CW = 128          # per-core feature slice width (2 heads x 64)
NCH = S // 128    # 16 chunks of 128 along the sequence

F32 = mybir.dt.float32
F32R = mybir.dt.float32r
EXP = mybir.ActivationFunctionType.Exp

_CACHE = {}


def _build_nc(rdt):
    nc = bacc.Bacc(
        "TRN2", target_bir_lowering=False, debug=False, enable_asserts=False
    )

    def dma_cast(dst_ap, src_ap):
        # pure bitcast on the DRAM side when loading into f32r tiles
        nc.sync.dma_start(dst_ap, src_ap.bitcast(rdt) if rdt != F32 else src_ap)

    Qs = nc.dram_tensor("Qs", [S, CW], F32, kind="ExternalInput")
    Ks = nc.dram_tensor("Ks", [S, CW], F32, kind="ExternalInput")
    Vs = nc.dram_tensor("Vs", [S, CW], F32, kind="ExternalInput")
    Wtq = nc.dram_tensor("Wtq", [CW, CW], F32, kind="ExternalInput")
    Wtk = nc.dram_tensor("Wtk", [CW, CW], F32, kind="ExternalInput")
    Wtv = nc.dram_tensor("Wtv", [CW, CW], F32, kind="ExternalInput")
    Bq = nc.dram_tensor("Bq", [CW, 1], F32, kind="ExternalInput")
    Bk = nc.dram_tensor("Bk", [CW, 1], F32, kind="ExternalInput")
    Bv0 = nc.dram_tensor("Bv0", [DH, 1], F32, kind="ExternalInput")
    Bv1 = nc.dram_tensor("Bv1", [DH, 1], F32, kind="ExternalInput")
    WoT = nc.dram_tensor("WoT", [CW, DK], F32, kind="ExternalInput")
    PT = nc.dram_tensor("PT", [DK, S], F32, kind="ExternalOutput")

    with tile.TileContext(nc) as tc:
        with ExitStack() as ctx:
            pers = ctx.enter_context(tc.tile_pool(name="pers", bufs=1))
            expool = ctx.enter_context(tc.tile_pool(name="expool", bufs=3))
            stage = ctx.enter_context(tc.tile_pool(name="stage", bufs=4))
            nrm = ctx.enter_context(tc.tile_pool(name="nrm", bufs=2))
            psmm = ctx.enter_context(tc.tile_pool(name="psmm", bufs=2, space="PSUM"))
            psacc = ctx.enter_context(tc.tile_pool(name="psacc", bufs=2, space="PSUM"))

            # ---- constants / weights ----
            ident = pers.tile([128, 128], F32, tag="ident")
            make_identity(nc, ident[:])

            wtq_sb = pers.tile([CW, CW], rdt, tag="wtq")
            wtk_sb = pers.tile([CW, CW], rdt, tag="wtk")
            wtv_sb = pers.tile([CW, CW], rdt, tag="wtv")
            dma_cast(wtq_sb[:], Wtq.ap())
            dma_cast(wtk_sb[:], Wtk.ap())
            dma_cast(wtv_sb[:], Wtv.ap())

            bq_sb = pers.tile([CW, 1], F32, tag="bq")
            bk_sb = pers.tile([CW, 1], F32, tag="bk")
            bv0_sb = pers.tile([DH, 1], F32, tag="bv0")
            bv1_sb = pers.tile([DH, 1], F32, tag="bv1")
            nc.sync.dma_start(bq_sb[:], Bq.ap())
            nc.sync.dma_start(bk_sb[:], Bk.ap())
            nc.sync.dma_start(bv0_sb[:], Bv0.ap())
            nc.sync.dma_start(bv1_sb[:], Bv1.ap())

            wo_sb = pers.tile([CW, DK], rdt, tag="wo")
            dma_cast(wo_sb[:], WoT.ap())

            # ---- load raw activation slices, chunk-major ----
            # raw[:, j*128:(j+1)*128] = X[j*128:(j+1)*128, :]
            qraw = pers.tile([128, S], F32, tag="qraw")
            kraw = pers.tile([128, S], F32, tag="kraw")
            vraw = pers.tile([128, S], F32, tag="vraw")
            for raw, dram in ((qraw, Qs), (kraw, Ks), (vraw, Vs)):
                nc.sync.dma_start(
                    raw[:].rearrange("p (j d) -> p j d", d=CW),
                    dram.ap().rearrange("(j p) d -> p j d", p=128),
                )

            # ---- PE-transpose raw chunks into T-space: XT[d2, s] ----
            QT = pers.tile([128, S], rdt, tag="QT")
            KT = pers.tile([128, S], rdt, tag="KT")
            VT = pers.tile([128, S], rdt, tag="VT")
            for raw, xt in ((qraw, QT), (kraw, KT), (vraw, VT)):
                for j in range(NCH):
                    pt = psmm.tile([128, 128], F32, tag="ps")
                    nc.tensor.transpose(pt[:], raw[:, ts(j, 128)], ident[:])
                    nc.vector.tensor_copy(xt[:, ts(j, 128)], pt[:])

            # ---- projections (T-space): xT = blockdiag(W.T) @ XT + b ----
            qTs = pers.tile([128, S], rdt, tag="qTs")
            kTs = pers.tile([128, S], rdt, tag="kTs")
            for wt, xt, xts, b in ((wtq_sb, QT, qTs, bq_sb), (wtk_sb, KT, kTs, bk_sb)):
                for sl in range(S // 512):
                    pp = psmm.tile([128, 512], F32, tag="ps")
                    nc.tensor.matmul(pp[:], wt[:], xt[:, ts(sl, 512)])
                    nc.vector.tensor_scalar_add(xts[:, ts(sl, 512)], pp[:], b[:])

            # v per head, with a ones row appended (softmax denominator trick)
            vaug = []
            for h, bvh in ((0, bv0_sb), (1, bv1_sb)):
                va = pers.tile([DH + 1, S], F32, tag=f"vaug{h}")
                nc.gpsimd.memset(va[DH : DH + 1, :], 1.0)
                for sl in range(S // 512):
                    pp = psmm.tile([DH, 512], F32, tag="ps")
                    nc.tensor.matmul(
                        pp[:], wtv_sb[:, ts(h, DH)], VT[:, ts(sl, 512)]
                    )
                    nc.vector.tensor_scalar_add(va[0:DH, ts(sl, 512)], pp[:], bvh[:])
                vaug.append(va)

            # ---- transpose v back to [t, e|1] chunks (stationary for attn@V) ----
            vS = []
            for h in (0, 1):
                vs = pers.tile([128, NCH * (DH + 1)], rdt, tag=f"vS{h}")
                for j in range(NCH):
                    pt = psmm.tile([128, DH + 1], F32, tag="ps")
                    nc.tensor.transpose(
                        pt[:],
                        vaug[h][:, ts(j, 128)],
                        ident[0 : DH + 1, 0 : DH + 1],
                    )
                    nc.vector.tensor_copy(vs[:, ts(j, DH + 1)], pt[:])
                vS.append(vs)

            # ---- attention, per head, per s-half ----
            # ones row at partition DH, used to PE-broadcast the reciprocal
            # softmax denominator (which lands on partition DH) to rows 0..63
            ones65_f = pers.tile([DH + 1, DH], F32, tag="ones65f")
            nc.gpsimd.memset(ones65_f[:], 1.0)
            if rdt == F32:
                ones65 = ones65_f
            else:
                ones65 = pers.tile([DH + 1, DH], rdt, tag="ones65")
                nc.vector.tensor_copy(ones65[:], ones65_f[:])
            oT_all = pers.tile([128, S], rdt, tag="oT")
            for h in (0, 1):
                hs = h * DH
                for sh in range(2):
                    s0 = sh * 1024
                    acc = psacc.tile([DH + 1, 1024], F32, tag="acc")
                    for j in range(NCH):
                        sc = psmm.tile([128, 1024], F32, tag="ps")
                        for n in range(2):
                            nc.tensor.matmul(
                                sc[:, ts(n, 512)],
                                kTs[hs : hs + DH, ts(j, 128)],
                                qTs[hs : hs + DH, s0 + n * 512 : s0 + (n + 1) * 512],
                            )
                        ex = expool.tile([128, 1024], rdt, tag="ex")
                        nc.scalar.activation(ex[:], sc[:], EXP, scale=0.125)
                        for n in range(2):
                            nc.tensor.matmul(
                                acc[:, ts(n, 512)],
                                vS[h][:, ts(j, DH + 1)],
                                ex[:, ts(n, 512)],
                                start=(j == 0),
                                stop=(j == NCH - 1),
                            )
                    # rows 0..63 are unnormalized o^T, row 64 the softmax denom
                    rc = nrm.tile([DH + 1, 1024], rdt, tag="rc")
                    with nc.allow_low_precision(
                        reason="f32r rounding of softmax reciprocal, ~1e-7"
                    ):
                        nc.vector.reciprocal(rc[DH : DH + 1, :], acc[DH : DH + 1, :])
                    bc = psmm.tile([DH, 1024], F32, tag="ps")
                    for n in range(2):
                        nc.tensor.matmul(
                            bc[:, ts(n, 512)],
                            ones65[DH : DH + 1, :],
                            rc[DH : DH + 1, ts(n, 512)],
                        )
                    rb = nrm.tile([DH, 1024], F32, tag="rb")
                    nc.vector.tensor_copy(rb[:], bc[:])
                    ot = nrm.tile([DH, 1024], rdt, tag="ot")
                    nc.vector.tensor_mul(ot[:], acc[0:DH, :], rb[:])
                    nc.sync.dma_start(oT_all[hs : hs + DH, s0 : s0 + 1024], ot[:])

            # ---- final linear (row-parallel partial): PT = WoT.T @ oT ----
            for mi in range(DK // 128):
                for ss in range(S // 512):
                    p4 = psmm.tile([128, 512], F32, tag="ps")
                    nc.tensor.matmul(
                        p4[:], wo_sb[:, ts(mi, 128)], oT_all[:, ts(ss, 512)]
                    )
                    st = stage.tile([128, 512], F32, tag="st")
                    nc.any.tensor_copy(st[:], p4[:])
                    nc.sync.dma_start(PT.ap()[ts(mi, 128), ts(ss, 512)], st[:])

    nc.compile()
    return nc


def _get_nc():
    if "nc" not in _CACHE:
        rdt = F32 if os.environ.get("MHA_PREC", "f32r") == "f32" else F32R
        _CACHE["nc"] = _build_nc(rdt)
    return _CACHE["nc"]


def make_in_maps(Q, K, V, Wq, bq, Wk, bk, Wv, bv, Wo):
    in_maps = []
    for i in range(NCORES):
        c0 = i * CW
        h0, h1 = 2 * i, 2 * i + 1

        def blockdiag_t(W):
            out = np.zeros((CW, CW), np.float32)
            out[0:DH, 0:DH] = W[h0].T
            out[DH:CW, DH:CW] = W[h1].T
            return out

        in_maps.append(
            {
                "Qs": np.ascontiguousarray(Q[:, c0 : c0 + CW]),
                "Ks": np.ascontiguousarray(K[:, c0 : c0 + CW]),
                "Vs": np.ascontiguousarray(V[:, c0 : c0 + CW]),
                "Wtq": blockdiag_t(Wq),
                "Wtk": blockdiag_t(Wk),
                "Wtv": blockdiag_t(Wv),
                "Bq": np.concatenate([bq[h0], bq[h1]]).reshape(CW, 1).astype(np.float32),
                "Bk": np.concatenate([bk[h0], bk[h1]]).reshape(CW, 1).astype(np.float32),
                "Bv0": bv[h0].reshape(DH, 1).astype(np.float32),
                "Bv1": bv[h1].reshape(DH, 1).astype(np.float32),
                "WoT": np.ascontiguousarray(Wo[:, c0 : c0 + CW].T),
            }
        )
    return in_maps


def kernel(Q, K, V, Wq, bq, Wk, bk, Wv, bv, Wo, bo, _spmd_kwargs=None):
    Q, K, V = (np.asarray(x, np.float32) for x in (Q, K, V))
    Wq, bq, Wk, bk, Wv, bv = (
        np.asarray(x, np.float32) for x in (Wq, bq, Wk, bk, Wv, bv)
    )
    Wo, bo = np.asarray(Wo, np.float32), np.asarray(bo, np.float32)

    nc = _get_nc()
    in_maps = make_in_maps(Q, K, V, Wq, bq, Wk, bk, Wv, bv, Wo)
    res = run_bass_kernel_spmd(
        nc, in_maps, core_ids=list(range(NCORES)), **(_spmd_kwargs or {})
    )

    # unshard: sum the row-parallel partials, add bo
    acc = np.zeros((DK, S), np.float64)
    for i in range(NCORES):
        acc += res.results[i]["PT"]
    out = (acc.T + bo).astype(np.float32)
    if _spmd_kwargs:
        return out, res
    return out


# revision 16
# speedup vs baseline: 1.1034x; 1.1034x over previous
"""Multi-head attention (16 heads, d_model=1024, S=2048) on 8 Trainium2 cores.

Sharding: tensor-parallel over heads — each core owns 2 heads (its slice of
Wq/Wk/Wv and the matching 128 columns of Q/K/V and of Wo).  Each core computes
its heads' attention and a row-parallel partial of the final linear; the host
sums the 8 partials and adds bo (the unshard step for row-parallel).

Device-side layout ("T-space"): activations are kept transposed, [feature,
seq], so that
  - projections contract the input feature dim (on partitions),
  - scores^T[t, s] = k_chunk.T @ qT needs no transpose of the attention matrix,
  - softmax denominators come free via a ones-column appended to V,
  - attn@V and the final linear consume exp(scores^T) chunks directly as the
    moving operand at full PE rate (f32r, N=512).
Only the raw Q/K/V input slices (and V again, post-projection, with the ones
row) are transposed, via cheap 128-wide PE transposes.  Softmax is computed
without max-subtraction: scores here are bounded (|s| < ~10), far from fp32
exp overflow, and softmax is shift-invariant.

Matmul operands use float32r (full-rate fp32 streaming, TF32-grade rounding,
~1e-4 relative per matmul).  Set MHA_PREC=f32 for exact-fp32 matmuls
(4x slower on the PE).
"""

import os
import sys

for _p in ("/opt/trn_rl_repo",):
    if _p not in sys.path:
        sys.path.insert(0, _p)

from contextlib import ExitStack

import numpy as np

import concourse.bass as bass
import concourse.tile as tile
from concourse import bacc, mybir
from concourse.bass import ts
from concourse.bass_utils import run_bass_kernel_spmd
from concourse.masks import make_identity

S = 2048          # sequence length
DK = 1024         # d_model
H = 16            # heads
DH = 64           # head dim
NCORES = 8
CW = 128          # per-core feature slice width (2 heads x 64)
NCH = S // 128    # 16 chunks of 128 along the sequence

F32 = mybir.dt.float32
F32R = mybir.dt.float32r
EXP = mybir.ActivationFunctionType.Exp

_CACHE = {}


def _build_nc(rdt):
    nc = bacc.Bacc(
        "TRN2", target_bir_lowering=False, debug=False, enable_asserts=False
    )

    def dma_cast(dst_ap, src_ap):
        # pure bitcast on the DRAM side when loading into f32r tiles
        nc.sync.dma_start(dst_ap, src_ap.bitcast(rdt) if rdt != F32 else src_ap)

    Qs = nc.dram_tensor("Qs", [S, CW], F32, kind="ExternalInput")
    Ks = nc.dram_tensor("Ks", [S, CW], F32, kind="ExternalInput")
    Vs = nc.dram_tensor("Vs", [S, CW], F32, kind="ExternalInput")
    Wtq = nc.dram_tensor("Wtq", [CW, CW], F32, kind="ExternalInput")
    Wtk = nc.dram_tensor("Wtk", [CW, CW], F32, kind="ExternalInput")
    Wtv = nc.dram_tensor("Wtv", [CW, CW], F32, kind="ExternalInput")
    Bq = nc.dram_tensor("Bq", [CW, 1], F32, kind="ExternalInput")
    Bk = nc.dram_tensor("Bk", [CW, 1], F32, kind="ExternalInput")
    Bv0 = nc.dram_tensor("Bv0", [DH, 1], F32, kind="ExternalInput")
    Bv1 = nc.dram_tensor("Bv1", [DH, 1], F32, kind="ExternalInput")
    WoT = nc.dram_tensor("WoT", [CW, DK], F32, kind="ExternalInput")
    PT = nc.dram_tensor("PT", [DK, S], F32, kind="ExternalOutput")

    with tile.TileContext(nc) as tc:
        with ExitStack() as ctx:
            pers = ctx.enter_context(tc.tile_pool(name="pers", bufs=1))
            expool = ctx.enter_context(tc.tile_pool(name="expool", bufs=3))
            stage = ctx.enter_context(tc.tile_pool(name="stage", bufs=4))
            nrm = ctx.enter_context(tc.tile_pool(name="nrm", bufs=2))
            psmm = ctx.enter_context(tc.tile_pool(name="psmm", bufs=2, space="PSUM"))
            psacc = ctx.enter_context(tc.tile_pool(name="psacc", bufs=2, space="PSUM"))
            dscr = ctx.enter_context(tc.tile_pool(name="dscr", bufs=2, space="DRAM"))

            # ---- constants / weights ----
            ident = pers.tile([128, 128], F32, tag="ident")
            make_identity(nc, ident[:])
            if rdt == F32:
                ident_r = ident
            else:
                ident_r = pers.tile([128, 128], rdt, tag="identr")
                nc.vector.tensor_copy(ident_r[:], ident[:])

            wtq_sb = pers.tile([CW, CW], rdt, tag="wtq")
            wtk_sb = pers.tile([CW, CW], rdt, tag="wtk")
            wtv_sb = pers.tile([CW, CW], rdt, tag="wtv")
            dma_cast(wtq_sb[:], Wtq.ap())
            dma_cast(wtk_sb[:], Wtk.ap())
            dma_cast(wtv_sb[:], Wtv.ap())

            bq_sb = pers.tile([CW, 1], F32, tag="bq")
            bk_sb = pers.tile([CW, 1], F32, tag="bk")
            bv0_sb = pers.tile([DH, 1], F32, tag="bv0")
            bv1_sb = pers.tile([DH, 1], F32, tag="bv1")
            nc.sync.dma_start(bq_sb[:], Bq.ap())
            nc.sync.dma_start(bk_sb[:], Bk.ap())
            nc.sync.dma_start(bv0_sb[:], Bv0.ap())
            nc.sync.dma_start(bv1_sb[:], Bv1.ap())

            wo_sb = pers.tile([CW, DK], rdt, tag="wo")
            dma_cast(wo_sb[:], WoT.ap())

            # ---- load raw activation slices, chunk-major ----
            # raw[:, j*128:(j+1)*128] = X[j*128:(j+1)*128, :]
            qraw = pers.tile([128, S], rdt, tag="qraw")
            kraw = pers.tile([128, S], rdt, tag="kraw")
            vraw = pers.tile([128, S], rdt, tag="vraw")
            for raw, dram in ((qraw, Qs), (kraw, Ks), (vraw, Vs)):
                src = dram.ap().bitcast(rdt) if rdt != F32 else dram.ap()
                nc.sync.dma_start(
                    raw[:].rearrange("p (j d) -> p j d", d=CW),
                    src.rearrange("(j p) d -> p j d", p=128),
                )

            # ---- PE-transpose raw chunks into T-space: XT[d2, s] ----
            # (f32r transpose mode: 1.5 cyc/row vs 2 for fp32)
            QT = pers.tile([128, S], rdt, tag="QT")
            KT = pers.tile([128, S], rdt, tag="KT")
            VT = pers.tile([128, S], rdt, tag="VT")
            for raw, xt in ((qraw, QT), (kraw, KT), (vraw, VT)):
                for j in range(NCH):
                    pt = psmm.tile([128, 128], rdt, tag="ps")
                    nc.tensor.transpose(pt[:], raw[:, ts(j, 128)], ident_r[:])
                    nc.vector.tensor_copy(xt[:, ts(j, 128)], pt[:])

            # ---- projections (T-space): xT = blockdiag(W.T) @ XT + b ----
            qTs = pers.tile([128, S], rdt, tag="qTs")
            kTs = pers.tile([128, S], rdt, tag="kTs")
            for wt, xt, xts, b in ((wtq_sb, QT, qTs, bq_sb), (wtk_sb, KT, kTs, bk_sb)):
                for sl in range(S // 512):
                    pp = psmm.tile([128, 512], F32, tag="ps")
                    nc.tensor.matmul(pp[:], wt[:], xt[:, ts(sl, 512)])
                    nc.vector.tensor_scalar_add(xts[:, ts(sl, 512)], pp[:], b[:])

            # v per head, with a ones row appended (softmax denominator trick)
            vaug = []
            for h, bvh in ((0, bv0_sb), (1, bv1_sb)):
                va = pers.tile([DH + 1, S], F32, tag=f"vaug{h}")
                nc.gpsimd.memset(va[DH : DH + 1, :], 1.0)
                for sl in range(S // 512):
                    pp = psmm.tile([DH, 512], F32, tag="ps")
                    nc.tensor.matmul(
                        pp[:], wtv_sb[:, ts(h, DH)], VT[:, ts(sl, 512)]
                    )
                    nc.vector.tensor_scalar_add(va[0:DH, ts(sl, 512)], pp[:], bvh[:])
                vaug.append(va)

            # ---- transpose v back to [t, e|1] chunks (stationary for attn@V) ----
            vS = []
            for h in (0, 1):
                vs = pers.tile([128, NCH * (DH + 1)], rdt, tag=f"vS{h}")
                for j in range(NCH):
                    pt = psmm.tile([128, DH + 1], F32, tag="ps")
                    nc.tensor.transpose(
                        pt[:],
                        vaug[h][:, ts(j, 128)],
                        ident[0 : DH + 1, 0 : DH + 1],
                    )
                    nc.vector.tensor_copy(vs[:, ts(j, DH + 1)], pt[:])
                vS.append(vs)

            # ---- attention, per head, per s-half ----
            oT_all = pers.tile([128, S], rdt, tag="oT")
            for h in (0, 1):
                hs = h * DH
                for sh in range(2):
                    s0 = sh * 1024
                    acc = psacc.tile([DH + 1, 1024], F32, tag="acc")
                    for j in range(NCH):
                        sc = psmm.tile([128, 1024], F32, tag="ps")
                        for n in range(2):
                            nc.tensor.matmul(
                                sc[:, ts(n, 512)],
                                kTs[hs : hs + DH, ts(j, 128)],
                                qTs[hs : hs + DH, s0 + n * 512 : s0 + (n + 1) * 512],
                            )
                        ex = expool.tile([128, 1024], rdt, tag="ex")
                        nc.scalar.activation(ex[:], sc[:], EXP, scale=0.125)
                        for n in range(2):
                            nc.tensor.matmul(
                                acc[:, ts(n, 512)],
                                vS[h][:, ts(j, DH + 1)],
                                ex[:, ts(n, 512)],
                                start=(j == 0),
                                stop=(j == NCH - 1),
                            )
                    # rows 0..63 are unnormalized o^T, row 64 the softmax denom.
                    # Copy the whole acc to SBUF (frees PSUM, all 65 lanes),
                    # bounce the denom row through DRAM to partition-broadcast
                    # it, then do the reciprocal on all 64 lanes in parallel.
                    # No PE involvement: the in-order PE queue never stalls on
                    # this chain.
                    oc = nrm.tile([DH + 1, 1024], F32, tag="oc")
                    nc.vector.tensor_copy(oc[:], acc[:])
                    dnd = dscr.tile([1, 1024], F32, tag="dnd")
                    nc.sync.dma_start(dnd[:], oc[DH : DH + 1, :])
                    db = nrm.tile([DH, 1024], F32, tag="db")
                    nc.sync.dma_start(db[:], dnd[0:1, :].to_broadcast((DH, 1024)))
                    rb = nrm.tile([DH, 1024], F32, tag="rb")
                    nc.vector.reciprocal(rb[:], db[:])
                    ot = nrm.tile([DH, 1024], rdt, tag="ot")
                    nc.vector.tensor_mul(ot[:], oc[0:DH, :], rb[:])
                    nc.sync.dma_start(oT_all[hs : hs + DH, s0 : s0 + 1024], ot[:])

            # ---- final linear (row-parallel partial): PT = WoT.T @ oT ----
            for mi in range(DK // 128):
                for ss in range(S // 512):
                    p4 = psmm.tile([128, 512], F32, tag="ps")
                    nc.tensor.matmul(
                        p4[:], wo_sb[:, ts(mi, 128)], oT_all[:, ts(ss, 512)]
                    )
                    st = stage.tile([128, 512], F32, tag="st")
                    # split the PSUM->SBUF drains across both free engines
                    if ss % 2 == 0:
                        nc.vector.tensor_copy(st[:], p4[:])
                    else:
                        nc.scalar.copy(st[:], p4[:])
                    nc.sync.dma_start(PT.ap()[ts(mi, 128), ts(ss, 512)], st[:])

    nc.compile()
    return nc


def _get_nc():
    if "nc" not in _CACHE:
        rdt = F32 if os.environ.get("MHA_PREC", "f32r") == "f32" else F32R
        _CACHE["nc"] = _build_nc(rdt)
    return _CACHE["nc"]


def make_in_maps(Q, K, V, Wq, bq, Wk, bk, Wv, bv, Wo):
    in_maps = []
    for i in range(NCORES):
        c0 = i * CW
        h0, h1 = 2 * i, 2 * i + 1

        def blockdiag_t(W):
            out = np.zeros((CW, CW), np.float32)
            out[0:DH, 0:DH] = W[h0].T
            out[DH:CW, DH:CW] = W[h1].T
            return out

        in_maps.append(
            {
                "Qs": np.ascontiguousarray(Q[:, c0 : c0 + CW]),
                "Ks": np.ascontiguousarray(K[:, c0 : c0 + CW]),
                "Vs": np.ascontiguousarray(V[:, c0 : c0 + CW]),
                "Wtq": blockdiag_t(Wq),
                "Wtk": blockdiag_t(Wk),
                "Wtv": blockdiag_t(Wv),
                "Bq": np.concatenate([bq[h0], bq[h1]]).reshape(CW, 1).astype(np.float32),
                "Bk": np.concatenate([bk[h0], bk[h1]]).reshape(CW, 1).astype(np.float32),
                "Bv0": bv[h0].reshape(DH, 1).astype(np.float32),
                "Bv1": bv[h1].reshape(DH, 1).astype(np.float32),
                "WoT": np.ascontiguousarray(Wo[:, c0 : c0 + CW].T),
            }
        )
    return in_maps


def kernel(Q, K, V, Wq, bq, Wk, bk, Wv, bv, Wo, bo, _spmd_kwargs=None):
    Q, K, V = (np.asarray(x, np.float32) for x in (Q, K, V))
    Wq, bq, Wk, bk, Wv, bv = (
        np.asarray(x, np.float32) for x in (Wq, bq, Wk, bk, Wv, bv)
    )
    Wo, bo = np.asarray(Wo, np.float32), np.asarray(bo, np.float32)

    nc = _get_nc()
    in_maps = make_in_maps(Q, K, V, Wq, bq, Wk, bk, Wv, bv, Wo)
    res = run_bass_kernel_spmd(
        nc, in_maps, core_ids=list(range(NCORES)), **(_spmd_kwargs or {})
    )

    # unshard: sum the row-parallel partials, add bo
    acc = np.zeros((DK, S), np.float64)
    for i in range(NCORES):
        acc += res.results[i]["PT"]
    out = (acc.T + bo).astype(np.float32)
    if _spmd_kwargs:
        return out, res
    return out
